# revision 5
# baseline (speedup 1.0000x reference)
"""FuzzyPooling Trainium2 kernel.

Computes y = avgpool2x2(x * exp(-x^2/2)) for x of shape (32, 64, 224, 224) f32,
output (32, 64, 112, 112) f32.

Sharding: pure data parallel over the batch dim — core c takes x[4c:4c+4].

Layout: with stride==kernel==2 pooling, each output row j of an image comes
from input rows 2j, 2j+1 — 448 contiguous floats in DRAM ("row-pair").  The
per-core tensor (4*64*224*224 elems) is 28672 row-pairs; a DMA chunk is
[128 partitions x 16 row-pairs] = [128 x 7168] f32 (28 KiB contiguous per
partition, 3.67 MB per transfer, 14 chunks/pass), and the pooled output chunk
[128 x 1792] f32 is exactly contiguous in the output tensor too — both DMAs
are pure reshapes of DRAM.

Math: exp(-x^2/2) = (sqrt(pi)/2) * d/dx erf(x/sqrt(2)), so the ACT engine's
Derivative_Erf table computes the gaussian in ONE pass (no Square pass), and
the constant (sqrt(pi)/2)*(1/4 pool mean) = sqrt(pi)/8 folds into the DVE
multiply.

Engine budget per core per pass (measured ~196 us, DMA-bound):
  DMA:  51.4 MB in + 12.85 MB out = 64.25 MB @ ~330 GB/s  (~195 us; the
        dma-only floor for this pattern measures 195-198 us vs 179 us HBM cap)
  in-DMA is SWDGE (gpsimd) with f32->bf16 cast-on-transfer; out-DMA HWDGE.
  ACT:  Derivative_Erf over 12.85M elems   ~88 us
  DVE (bf16, 2x/cycle tensor_tensor):  m=(e*K)*x, row-add, col-add  ~110 us
Accuracy: bf16 intermediates give rel_err ~3.4e-3 (vs 2e-2 gate).
"""

import math

import numpy as np

import concourse.bass as bass  # noqa: F401
import concourse.mybir as mybir
from concourse import bacc, tile
from concourse.bass_utils import run_bass_kernel_spmd

AF = mybir.ActivationFunctionType
ALU = mybir.AluOpType

N_CORES = 8
B, C, H, W = 32, 64, 224, 224
OH, OW = H // 2, W // 2
B_PER_CORE = B // N_CORES              # 4
ROWPAIRS = B_PER_CORE * C * OH         # 28672 row-pairs of 448 f32 per core
RP_C = 16                              # row-pairs per partition per DMA chunk
IN_F = RP_C * 2 * W                    # 7168 f32 per partition per chunk
OUT_F = RP_C * OW                      # 1792 f32
NCHUNK = ROWPAIRS // (128 * RP_C)      # 14
K = math.sqrt(math.pi) / 8.0           # (sqrt(pi)/2) [dErf] * (1/4) [mean]
S2 = 1.0 / math.sqrt(2.0)
OB = 2                                 # output chunks batched per out-DMA
BUFS = (4, 4, 4, 3)

_CACHE = {}


def _emit_chunk(nc, ch, x, out, pools, state):
    f32, bf16 = mybir.dt.float32, mybir.dt.bfloat16
    xpool, epool, vpool, opool = pools
    xt = xpool.tile([128, IN_F], bf16, tag="xt")
    nc.gpsimd.dma_start(out=xt[:], in_=x[ch])   # SWDGE: cast f32->bf16 on DMA
    et = epool.tile([128, IN_F], bf16, tag="et")
    # e = dErf(x/sqrt2) = (2/sqrt(pi)) exp(-x^2/2)
    nc.scalar.activation(et[:], xt[:], AF.Derivative_Erf, scale=S2)
    # m = (e * K) * x = x exp(-x^2/2) / 4   (in place over et)
    nc.vector.scalar_tensor_tensor(out=et[:], in0=et[:], scalar=K, in1=xt[:],
                                   op0=ALU.mult, op1=ALU.mult)
    mv = et[:].rearrange("p (k t w) -> p k t w", k=RP_C, t=2)
    v = vpool.tile([128, IN_F // 2], bf16, tag="v")
    vv = v[:].rearrange("p (k w) -> p k w", k=RP_C)
    nc.vector.tensor_tensor(out=vv, in0=mv[:, :, 0, :], in1=mv[:, :, 1, :],
                            op=ALU.add)
    vp = v[:].rearrange("p (k w t) -> p k w t", k=RP_C, t=2)
    if ch % OB == 0:
        o = opool.tile([128, OUT_F * OB], f32, tag="o")
        state["o"] = o
    o = state["o"]
    off = (ch % OB) * OUT_F
    ov = o[:, off:off + OUT_F].rearrange("p (k w) -> p k w", k=RP_C)
    nc.vector.tensor_tensor(out=ov, in0=vp[:, :, :, 0], in1=vp[:, :, :, 1],
                            op=ALU.add)
    if ch % OB == OB - 1:
        # one out-DMA per OB chunks: 7x 1.83MB instead of 14x 0.92MB —
        # fewer HBM-write completion receipts (~4us/pass on HW)
        dst = out[ch - OB + 1:ch + 1].rearrange("c p f -> p c f")
        src = o[:].rearrange("p (c f) -> p c f", c=OB)
        nc.scalar.dma_start(out=dst, in_=src)


def _build_nc():
    f32 = mybir.dt.float32
    nc = bacc.Bacc("TRN2", target_bir_lowering=False, debug=False,
                   num_devices=N_CORES)
    x = nc.dram_tensor("x", [NCHUNK, 128, IN_F], f32,
                       kind="ExternalInput").ap()
    out = nc.dram_tensor("out", [NCHUNK, 128, OUT_F], f32,
                         kind="ExternalOutput").ap()
    with tile.TileContext(nc) as tc:
        with tc.tile_pool(name="xin", bufs=BUFS[0]) as xpool, \
             tc.tile_pool(name="e", bufs=BUFS[1]) as epool, \
             tc.tile_pool(name="v", bufs=BUFS[2]) as vpool, \
             tc.tile_pool(name="o", bufs=BUFS[3]) as opool:
            pools = (xpool, epool, vpool, opool)
            state = {}
            for ch in range(NCHUNK):
                _emit_chunk(nc, ch, x, out, pools, state)
    nc.compile()
    return nc


def _get_nc():
    if "nc" not in _CACHE:
        _CACHE["nc"] = _build_nc()
    return _CACHE["nc"]


def _run(x: np.ndarray, trace: bool = False):
    nc = _get_nc()
    in_maps = []
    for c in range(N_CORES):
        shard = np.ascontiguousarray(x[c * B_PER_CORE:(c + 1) * B_PER_CORE])
        in_maps.append({"x": shard.reshape(NCHUNK, 128, IN_F)})
    res = run_bass_kernel_spmd(nc, in_maps, core_ids=list(range(N_CORES)),
                               trace=trace)
    parts = [r["out"].reshape(B_PER_CORE, C, OH, OW) for r in res.results]
    return np.concatenate(parts, axis=0), res


def kernel(x: np.ndarray) -> np.ndarray:
    out, _ = _run(np.asarray(x, dtype=np.float32), trace=False)
    return out
